# revision 11
# baseline (speedup 1.0000x reference)
"""Trainium2 Bass kernel for nn_MimicNetLSTM (2-layer LSTM, H=4096, batch=1, seq=1).

Strategy (tensor-parallel over the 4H gate dim, 8 cores):
  - Core r owns h-indices [512r, 512r+512) of every gate -> 2048 rows of each
    of w_ih0/w_hh0/w_ih1/w_hh1 (~105 MB fp32 per core).  The problem is a
    batch-1 matvec chain, so it is HBM-bandwidth bound: stream weights once.
  - Host pre-arranges each core's weight shard as [K, 2048] (transposed), with
    rows permuted so that contraction chunk c / partition p corresponds to the
    SBUF layout of the activation vector (partition-major reshape).
  - PE computes gates with weights stationary: for each k-chunk (128) and each
    j-tile t (16), matmul(psum[:, t], lhsT=wt[:, 128t:128t+128], rhs=x[:, c])
    accumulating over k-chunks.  Gates land in PSUM as [128, 16] =
    [i(4 cols) | f | g | o] with h_local = 128*u + m  (column t = 4g+u).
  - LSTM pointwise on DVE/ACT in that [128, 4]-per-gate layout.
  - h1 (512 floats/core) is AllGathered between layers (hidden under the
    layer-1 weight stream).  Heads are computed as per-core partial dot
    products, AllGathered (8 B/core) and summed on every core.
"""

import os
import numpy as np

import concourse.bass as bass
import concourse.tile as tile
from concourse import bacc, mybir
from concourse.bass_utils import run_bass_kernel_spmd

I, H, L = 512, 4096, 2
NC = 8
SH = H // NC          # 512 h-indices per core
RJ = 4 * SH           # 2048 gate rows per core
FD = mybir.dt.float32

# weight dtype on the wire (HBM) + in the matmul. float32 = exact;
# bfloat16 halves DMA bytes (~2x faster) at ~3e-3 relative error.
WEIGHT_DTYPE = os.environ.get("KERNEL_WDT", "float32")

LAST_EXEC_NS = None
LAST_RESULTS = None


def _wdt():
    return getattr(mybir.dt, WEIGHT_DTYPE)


def _np_wdt():
    if WEIGHT_DTYPE == "float32":
        return np.float32
    import ml_dtypes

    return getattr(ml_dtypes, WEIGHT_DTYPE)


def _build_program(dbg=False):
    nc = bacc.Bacc(
        "TRN2",
        target_bir_lowering=False,
        debug=False,
        enable_asserts=False,
        num_devices=NC,
    )
    wdt = _wdt()

    wih0 = nc.dram_tensor("wih0", [I, RJ], wdt, kind="ExternalInput")
    whh0 = nc.dram_tensor("whh0", [H, RJ], wdt, kind="ExternalInput")
    whh1 = nc.dram_tensor("whh1", [H, RJ], wdt, kind="ExternalInput")
    wih1 = nc.dram_tensor("wih1", [H, RJ], wdt, kind="ExternalInput")
    x_in = nc.dram_tensor("x_in", [128, I // 128], wdt, kind="ExternalInput")
    h00 = nc.dram_tensor("h00", [128, H // 128], wdt, kind="ExternalInput")
    h01 = nc.dram_tensor("h01", [128, H // 128], wdt, kind="ExternalInput")
    c00 = nc.dram_tensor("c00", [128, 4], FD, kind="ExternalInput")
    c01 = nc.dram_tensor("c01", [128, 4], FD, kind="ExternalInput")
    b0 = nc.dram_tensor("b0", [128, 16], FD, kind="ExternalInput")
    b1 = nc.dram_tensor("b1", [128, 16], FD, kind="ExternalInput")
    wld = nc.dram_tensor("wld", [128, 8], FD, kind="ExternalInput")
    b2 = nc.dram_tensor("b2", [2, 1], FD, kind="ExternalInput")
    out_l = nc.dram_tensor("out_l", [1, 1], FD, kind="ExternalOutput")
    out_d = nc.dram_tensor("out_d", [1, 1], FD, kind="ExternalOutput")
    if dbg:
        dbg_g0 = nc.dram_tensor("dbg_g0", [128, 16], FD, kind="ExternalOutput")
        dbg_h1 = nc.dram_tensor("dbg_h1", [128, 4], FD, kind="ExternalOutput")
        dbg_h1f = nc.dram_tensor("dbg_h1f", [128, 32], FD, kind="ExternalOutput")
        dbg_g1 = nc.dram_tensor("dbg_g1", [128, 16], FD, kind="ExternalOutput")
        dbg_h2 = nc.dram_tensor("dbg_h2", [128, 4], FD, kind="ExternalOutput")
        dbg_hd = nc.dram_tensor("dbg_hd", [2, 1], FD, kind="ExternalOutput")

    with tile.TileContext(nc) as tc:
        with (
            tc.tile_pool(name="w", bufs=8) as wpool,
            tc.tile_pool(name="small", bufs=1) as small,
            tc.tile_pool(name="pw", bufs=2) as pw,
            tc.tile_pool(name="psum", bufs=1, space="PSUM") as ppool,
            tc.tile_pool(name="dram", bufs=1, space="DRAM") as dram,
        ):
            def load_small(name, src, shape, dtype=FD):
                t = small.tile(shape, dtype, tag=name)
                nc.sync.dma_start(t[:], src[:])
                return t

            x_sb = load_small("x", x_in, [128, I // 128], wdt)
            h00_sb = load_small("h00", h00, [128, H // 128], wdt)
            h01_sb = load_small("h01", h01, [128, H // 128], wdt)
            c00_sb = load_small("c00", c00, [128, 4])
            c01_sb = load_small("c01", c01, [128, 4])
            b0_sb = load_small("b0", b0, [128, 16])
            b1_sb = load_small("b1", b1, [128, 16])
            wld_sb = load_small("wld", wld, [128, 8])
            b2_sb = load_small("b2", b2, [2, 1])
            ones8 = small.tile([8, 1], FD, tag="ones8")
            nc.vector.memset(ones8[:], 1.0)

            psum_g0 = ppool.tile([128, 16], FD, tag="g0")
            psum_g1 = ppool.tile([128, 16], FD, tag="g1")

            def mm_stream(wdram, rhs_sb, psum, kchunks, first, last):
                for c in range(kchunks):
                    wt = wpool.tile([128, RJ], wdt, tag="w")
                    nc.sync.dma_start(wt[:], wdram[c * 128:(c + 1) * 128, :])
                    for t in range(16):
                        nc.tensor.matmul(
                            psum[:, t:t + 1],
                            lhsT=wt[:, t * 128:(t + 1) * 128],
                            rhs=rhs_sb[:, c:c + 1],
                            # start=True clears the whole PSUM bank, so it
                            # must only be set on the first matmul into it
                            start=(first and c == 0 and t == 0),
                            stop=(last and c == kchunks - 1 and t == 15),
                        )

            def pointwise(psum_g, bias_sb, c_sb):
                SIG = mybir.ActivationFunctionType.Sigmoid
                TANH = mybir.ActivationFunctionType.Tanh
                gb = pw.tile([128, 16], FD, tag="gb")
                nc.vector.tensor_add(gb[:], psum_g[:], bias_sb[:])
                act = pw.tile([128, 16], FD, tag="act")
                nc.scalar.activation(act[:, 0:8], gb[:, 0:8], SIG)     # i, f
                nc.scalar.activation(act[:, 12:16], gb[:, 12:16], SIG)  # o
                nc.scalar.activation(act[:, 8:12], gb[:, 8:12], TANH)   # g
                t1 = pw.tile([128, 4], FD, tag="t1")
                nc.vector.tensor_mul(t1[:], act[:, 4:8], c_sb[:])
                t2 = pw.tile([128, 4], FD, tag="t2")
                nc.vector.tensor_mul(t2[:], act[:, 0:4], act[:, 8:12])
                cn = pw.tile([128, 4], FD, tag="cn")
                nc.vector.tensor_add(cn[:], t1[:], t2[:])
                th = pw.tile([128, 4], FD, tag="th")
                nc.scalar.activation(th[:], cn[:], TANH)
                hn = pw.tile([128, 4], FD, tag="hn")
                nc.vector.tensor_mul(hn[:], act[:, 12:16], th[:])
                return hn

            # ---- layer 0 ----
            mm_stream(wih0, x_sb, psum_g0, I // 128, first=True, last=False)
            mm_stream(whh0, h00_sb, psum_g0, H // 128, first=False, last=True)
            h1_sb = pointwise(psum_g0, b0_sb, c00_sb)
            if dbg:
                g0_sb = pw.tile([128, 16], FD, tag="dbg_g0")
                nc.vector.tensor_copy(g0_sb[:], psum_g0[:])
                nc.sync.dma_start(dbg_g0[:], g0_sb[:])
                nc.sync.dma_start(dbg_h1[:], h1_sb[:])

            # AllGather h1: 512 floats/core -> 4096
            ag_in = dram.tile([128, 4], FD, tag="ag_in")
            nc.sync.dma_start(ag_in[:], h1_sb[:])
            ag_out = dram.tile([128, 32], FD, tag="ag_out")
            nc.gpsimd.collective_compute(
                "AllGather",
                mybir.AluOpType.bypass,
                replica_groups=[list(range(NC))],
                ins=[ag_in.opt()],
                outs=[ag_out.opt()],
            )
            h1f_sb = small.tile([128, 32], FD, tag="h1f")
            nc.sync.dma_start(h1f_sb[:], ag_out[:])
            if _wdt() != FD:
                h1c_sb = small.tile([128, 32], _wdt(), tag="h1c")
                nc.vector.tensor_copy(h1c_sb[:], h1f_sb[:])
            else:
                h1c_sb = h1f_sb

            # ---- layer 1 ----  (whh1 first: it doesn't depend on the AllGather)
            mm_stream(whh1, h01_sb, psum_g1, H // 128, first=True, last=False)
            mm_stream(wih1, h1c_sb, psum_g1, H // 128, first=False, last=True)
            h2_sb = pointwise(psum_g1, b1_sb, c01_sb)
            if dbg:
                nc.sync.dma_start(dbg_h1f[:], h1f_sb[:])
                g1_sb = pw.tile([128, 16], FD, tag="dbg_g1")
                nc.vector.tensor_copy(g1_sb[:], psum_g1[:])
                nc.sync.dma_start(dbg_g1[:], g1_sb[:])
                nc.sync.dma_start(dbg_h2[:], h2_sb[:])

            # ---- heads: partial dots on this core's 512 h-indices ----
            psum_hd = ppool.tile([2, 1], FD, tag="hd")
            for u in range(4):
                nc.tensor.matmul(
                    psum_hd[:, :],
                    lhsT=wld_sb[:, 2 * u:2 * u + 2],
                    rhs=h2_sb[:, u:u + 1],
                    start=(u == 0),
                    stop=(u == 3),
                )
            part_sb = pw.tile([2, 1], FD, tag="part")
            nc.vector.tensor_copy(part_sb[:], psum_hd[:])
            if dbg:
                nc.sync.dma_start(dbg_hd[:], part_sb[:])

            pd_in = dram.tile([2, 1], FD, tag="pd_in")
            nc.sync.dma_start(pd_in[:], part_sb[:])
            pd_out = dram.tile([8, 2], FD, tag="pd_out")
            nc.gpsimd.collective_compute(
                "AllGather",
                mybir.AluOpType.bypass,
                replica_groups=[list(range(NC))],
                ins=[pd_in.opt()],
                outs=[pd_out.opt()],
            )
            agp_sb = small.tile([8, 2], FD, tag="agp")
            nc.sync.dma_start(agp_sb[:], pd_out[:])

            psum_f = ppool.tile([2, 1], FD, tag="fin")
            nc.tensor.matmul(
                psum_f[:, :], lhsT=agp_sb[:, :], rhs=ones8[:, :],
                start=True, stop=True,
            )
            fin_sb = pw.tile([2, 1], FD, tag="fin_sb")
            nc.vector.tensor_add(fin_sb[:], psum_f[:], b2_sb[:])
            sig_sb = pw.tile([2, 1], FD, tag="sig_sb")
            nc.scalar.activation(
                sig_sb[:], fin_sb[:],
                mybir.ActivationFunctionType.Sigmoid,
            )
            nc.sync.dma_start(out_l[:], fin_sb[0:1, :])
            nc.sync.dma_start(out_d[:], sig_sb[1:2, :])

    nc.compile()
    return nc


_PROGRAM = None


def _get_program():
    global _PROGRAM
    if _PROGRAM is None:
        _PROGRAM = _build_program(
            dbg=bool(int(os.environ.get("KERNEL_DEBUG", "0"))))
    return _PROGRAM


def make_in_maps(data, h0, c0, w_ih0, w_hh0, b_ih0, b_hh0,
                 w_ih1, w_hh1, b_ih1, b_hh1, wL, bL, wD, bD):
    """Shard + lay out the full inputs for the 8 cores."""
    f32 = np.float32
    data, h0, c0 = (np.asarray(a, f32) for a in (data, h0, c0))
    w_ih0, w_hh0, w_ih1, w_hh1 = (
        np.asarray(a, f32) for a in (w_ih0, w_hh0, w_ih1, w_hh1))
    btot0 = np.asarray(b_ih0, f32) + np.asarray(b_hh0, f32)
    btot1 = np.asarray(b_ih1, f32) + np.asarray(b_hh1, f32)
    wL, bL, wD, bD = (np.asarray(a, f32) for a in (wL, bL, wD, bD))
    wdt = _np_wdt()

    p = np.arange(128)
    # contraction slot (c*128 + p) <-> true index, for partition-major rhs
    ordx = (4 * p[None, :] + np.arange(4)[:, None]).reshape(-1)        # I=512
    ordh = (32 * p[None, :] + np.arange(32)[:, None]).reshape(-1)      # H=4096
    # layer-1 W_ih contraction follows the AllGather buffer order:
    # AG position q = 512*r + 4*m + u  holds true h-index 512*r + 128*u + m
    q = ordh
    rem = q % 512
    ord_l1 = 512 * (q // 512) + 128 * (rem % 4) + rem // 4

    x_c = np.ascontiguousarray(data.reshape(128, 4), dtype=wdt)
    h00_c = np.ascontiguousarray(h0[0, 0].reshape(128, 32), dtype=wdt)
    h01_c = np.ascontiguousarray(h0[1, 0].reshape(128, 32), dtype=wdt)
    b2_c = np.array([[bL[0]], [bD[0]]], f32)

    in_maps = []
    for r in range(NC):
        rows = np.concatenate([g * H + SH * r + np.arange(SH) for g in range(4)])
        sl = slice(SH * r, SH * (r + 1))
        wld_c = np.empty((128, 8), f32)
        wld_c[:, 0::2] = wL[0, sl].reshape(4, 128).T
        wld_c[:, 1::2] = wD[0, sl].reshape(4, 128).T
        in_maps.append({
            "wih0": np.ascontiguousarray(w_ih0[rows].T[ordx], dtype=wdt),
            "whh0": np.ascontiguousarray(w_hh0[rows].T[ordh], dtype=wdt),
            "whh1": np.ascontiguousarray(w_hh1[rows].T[ordh], dtype=wdt),
            "wih1": np.ascontiguousarray(w_ih1[rows].T[ord_l1], dtype=wdt),
            "x_in": x_c,
            "h00": h00_c,
            "h01": h01_c,
            "c00": np.ascontiguousarray(c0[0, 0, sl].reshape(4, 128).T),
            "c01": np.ascontiguousarray(c0[1, 0, sl].reshape(4, 128).T),
            "b0": np.ascontiguousarray(btot0[rows].reshape(16, 128).T),
            "b1": np.ascontiguousarray(btot1[rows].reshape(16, 128).T),
            "wld": wld_c,
            "b2": b2_c,
        })
    return in_maps


def kernel(**inputs):
    global LAST_EXEC_NS
    in_maps = make_in_maps(**inputs)
    nc = _get_program()
    trace = bool(int(os.environ.get("KERNEL_TRACE", "0")))
    res = run_bass_kernel_spmd(
        nc, in_maps, core_ids=list(range(NC)), trace=trace,
    )
    LAST_EXEC_NS = res.exec_time_ns
    global LAST_RESULTS
    LAST_RESULTS = res.results
    r0 = res.results[0]
    d = np.asarray(r0["out_d"], np.float32).reshape(1, 1)
    l = np.asarray(r0["out_l"], np.float32).reshape(1, 1)
    return (d, l)


# revision 12
# speedup vs baseline: 1.8281x; 1.8281x over previous
"""Trainium2 Bass kernel for nn_MimicNetLSTM (2-layer LSTM, H=4096, batch=1, seq=1).

Strategy (tensor-parallel over the 4H gate dim, 8 cores):
  - Core r owns h-indices [512r, 512r+512) of every gate -> 2048 rows of each
    of w_ih0/w_hh0/w_ih1/w_hh1 (~105 MB fp32 per core).  The problem is a
    batch-1 matvec chain, so it is HBM-bandwidth bound: stream weights once.
  - Host pre-arranges each core's weight shard transposed as [K, 2048], rows
    permuted so contraction chunk c / partition p matches the partition-major
    SBUF layout of the activation vector (x/h reshaped [128, K/128]).
  - PE matvec with the ACTIVATION as the stationary operand (1-column
    LDWEIGHTS) and the weight tiles as the moving operand (N=512):
      psum[0:1, n*512:(n+1)*512] += x[:, c].T @ wt[:, n*512:(n+1)*512]
    accumulated over k-chunks c.  Gates land in PSUM partition 0 as
    [1, 2048] = [i | f | g | o] in true h order.
  - LSTM pointwise on DVE/ACT on partition 0.
  - h1 (512 floats/core) is AllGathered between layers (hidden under the
    layer-1 weight stream).  Heads are per-core partial dot products (DVE
    mul + reduce), AllGathered (8 B/core) and summed on every core.
"""

import os
import numpy as np

import concourse.bass as bass
import concourse.tile as tile
from concourse import bacc, mybir
from concourse.bass_utils import run_bass_kernel_spmd

I, H, L = 512, 4096, 2
NC = 8
SH = H // NC          # 512 h-indices per core
RJ = 4 * SH           # 2048 gate rows per core
FD = mybir.dt.float32

# weight dtype on the wire (HBM) + in the matmul. float32 = exact;
# bfloat16 halves DMA bytes (~2x faster) at ~3e-3 relative error.
WEIGHT_DTYPE = os.environ.get("KERNEL_WDT", "float32")

LAST_EXEC_NS = None
LAST_RESULTS = None


def _wdt():
    return getattr(mybir.dt, WEIGHT_DTYPE)


def _np_wdt():
    if WEIGHT_DTYPE == "float32":
        return np.float32
    import ml_dtypes

    return getattr(ml_dtypes, WEIGHT_DTYPE)


def _build_program(dbg=False):
    nc = bacc.Bacc(
        "TRN2",
        target_bir_lowering=False,
        debug=False,
        enable_asserts=False,
        num_devices=NC,
    )
    wdt = _wdt()

    wih0 = nc.dram_tensor("wih0", [I, RJ], wdt, kind="ExternalInput")
    whh0 = nc.dram_tensor("whh0", [H, RJ], wdt, kind="ExternalInput")
    whh1 = nc.dram_tensor("whh1", [H, RJ], wdt, kind="ExternalInput")
    wih1 = nc.dram_tensor("wih1", [H, RJ], wdt, kind="ExternalInput")
    x_in = nc.dram_tensor("x_in", [128, I // 128], wdt, kind="ExternalInput")
    h00 = nc.dram_tensor("h00", [128, H // 128], wdt, kind="ExternalInput")
    h01 = nc.dram_tensor("h01", [128, H // 128], wdt, kind="ExternalInput")
    c00 = nc.dram_tensor("c00", [1, SH], FD, kind="ExternalInput")
    c01 = nc.dram_tensor("c01", [1, SH], FD, kind="ExternalInput")
    b0 = nc.dram_tensor("b0", [1, RJ], FD, kind="ExternalInput")
    b1 = nc.dram_tensor("b1", [1, RJ], FD, kind="ExternalInput")
    wld = nc.dram_tensor("wld", [1, 2 * SH], FD, kind="ExternalInput")
    b2 = nc.dram_tensor("b2", [2, 1], FD, kind="ExternalInput")
    out_l = nc.dram_tensor("out_l", [1, 1], FD, kind="ExternalOutput")
    out_d = nc.dram_tensor("out_d", [1, 1], FD, kind="ExternalOutput")
    if dbg:
        dbg_g0 = nc.dram_tensor("dbg_g0", [1, RJ], FD, kind="ExternalOutput")
        dbg_h1 = nc.dram_tensor("dbg_h1", [1, SH], FD, kind="ExternalOutput")
        dbg_h1f = nc.dram_tensor("dbg_h1f", [128, 32], FD, kind="ExternalOutput")
        dbg_g1 = nc.dram_tensor("dbg_g1", [1, RJ], FD, kind="ExternalOutput")
        dbg_h2 = nc.dram_tensor("dbg_h2", [1, SH], FD, kind="ExternalOutput")
        dbg_hd = nc.dram_tensor("dbg_hd", [1, 2], FD, kind="ExternalOutput")

    SIG = mybir.ActivationFunctionType.Sigmoid
    TANH = mybir.ActivationFunctionType.Tanh

    with tile.TileContext(nc) as tc:
        with (
            tc.tile_pool(name="w", bufs=8) as wpool,
            tc.tile_pool(name="small", bufs=1) as small,
            tc.tile_pool(name="pw", bufs=2) as pw,
            tc.tile_pool(name="psum", bufs=1, space="PSUM") as ppool,
            tc.tile_pool(name="dram", bufs=1, space="DRAM") as dram,
        ):
            def load_small(name, src, shape, dtype=FD):
                t = small.tile(shape, dtype, tag=name)
                nc.sync.dma_start(t[:], src[:])
                return t

            x_sb = load_small("x", x_in, [128, I // 128], wdt)
            h00_sb = load_small("h00", h00, [128, H // 128], wdt)
            h01_sb = load_small("h01", h01, [128, H // 128], wdt)
            c00_sb = load_small("c00", c00, [1, SH])
            c01_sb = load_small("c01", c01, [1, SH])
            b0_sb = load_small("b0", b0, [1, RJ])
            b1_sb = load_small("b1", b1, [1, RJ])
            wld_sb = load_small("wld", wld, [1, 2 * SH])
            b2_sb = load_small("b2", b2, [2, 1])
            ones8 = small.tile([8, 1], FD, tag="ones8")
            nc.vector.memset(ones8[:], 1.0)

            def mm_stream(wdram, rhs_sb, psum, kchunks, first, last):
                for c in range(kchunks):
                    wt = wpool.tile([128, RJ], wdt, tag="w")
                    nc.sync.dma_start(wt[:], wdram[c * 128:(c + 1) * 128, :])
                    for n in range(4):
                        nc.tensor.matmul(
                            psum[0:1, n * 512:(n + 1) * 512],
                            lhsT=rhs_sb[:, c:c + 1],        # stationary (1 col)
                            rhs=wt[:, n * 512:(n + 1) * 512],  # moving weights
                            # each n-slice is its own PSUM bank; start clears
                            # the whole bank so set it on the bank's first MM
                            start=(first and c == 0),
                            stop=(last and c == kchunks - 1),
                        )

            def pointwise(psum_g, bias_sb, c_sb):
                # everything on partition 0; slices of [1, 2048] = [i|f|g|o]
                gb = pw.tile([1, RJ], FD, tag="gb")
                nc.vector.tensor_add(gb[:], psum_g[0:1, :], bias_sb[:])
                act = pw.tile([1, RJ], FD, tag="act")
                nc.scalar.activation(act[0:1, 0:2 * SH], gb[0:1, 0:2 * SH], SIG)
                nc.scalar.activation(act[0:1, 3 * SH:], gb[0:1, 3 * SH:], SIG)
                nc.scalar.activation(
                    act[0:1, 2 * SH:3 * SH], gb[0:1, 2 * SH:3 * SH], TANH)
                t1 = pw.tile([1, SH], FD, tag="t1")
                nc.vector.tensor_mul(t1[:], act[0:1, SH:2 * SH], c_sb[:])
                t2 = pw.tile([1, SH], FD, tag="t2")
                nc.vector.tensor_mul(
                    t2[:], act[0:1, 0:SH], act[0:1, 2 * SH:3 * SH])
                cn = pw.tile([1, SH], FD, tag="cn")
                nc.vector.tensor_add(cn[:], t1[:], t2[:])
                th = pw.tile([1, SH], FD, tag="th")
                nc.scalar.activation(th[:], cn[:], TANH)
                hn = pw.tile([1, SH], FD, tag="hn")
                nc.vector.tensor_mul(hn[:], act[0:1, 3 * SH:], th[:])
                return hn

            # ---- layer 0 ----
            psum_g0 = ppool.tile([1, RJ], FD, tag="g")
            mm_stream(wih0, x_sb, psum_g0, I // 128, first=True, last=False)
            mm_stream(whh0, h00_sb, psum_g0, H // 128, first=False, last=True)
            h1_sb = pointwise(psum_g0, b0_sb, c00_sb)
            if dbg:
                g0_sb = pw.tile([1, RJ], FD, tag="dbg_g0")
                nc.vector.tensor_copy(g0_sb[:], psum_g0[0:1, :])
                nc.sync.dma_start(dbg_g0[:], g0_sb[:])
                nc.sync.dma_start(dbg_h1[:], h1_sb[:])

            # AllGather h1: 512 floats/core -> 4096 (true h order)
            ag_in = dram.tile([1, SH], FD, tag="ag_in")
            nc.sync.dma_start(ag_in[:], h1_sb[:])
            ag_out = dram.tile([128, 32], FD, tag="ag_out")
            nc.gpsimd.collective_compute(
                "AllGather",
                mybir.AluOpType.bypass,
                replica_groups=[list(range(NC))],
                ins=[ag_in.opt()],
                outs=[ag_out.opt()],
            )
            h1f_sb = small.tile([128, 32], FD, tag="h1f")
            nc.sync.dma_start(h1f_sb[:], ag_out[:])
            if _wdt() != FD:
                h1c_sb = small.tile([128, 32], _wdt(), tag="h1c")
                nc.vector.tensor_copy(h1c_sb[:], h1f_sb[:])
            else:
                h1c_sb = h1f_sb

            # ---- layer 1 ----  (whh1 first: it doesn't depend on the AllGather;
            # psum tag "g" is reused -> waits only for pointwise0's psum read)
            psum_g1 = ppool.tile([1, RJ], FD, tag="g")
            mm_stream(whh1, h01_sb, psum_g1, H // 128, first=True, last=False)
            mm_stream(wih1, h1c_sb, psum_g1, H // 128, first=False, last=True)
            h2_sb = pointwise(psum_g1, b1_sb, c01_sb)
            if dbg:
                nc.sync.dma_start(dbg_h1f[:], h1f_sb[:])
                g1_sb = pw.tile([1, RJ], FD, tag="dbg_g1")
                nc.vector.tensor_copy(g1_sb[:], psum_g1[0:1, :])
                nc.sync.dma_start(dbg_g1[:], g1_sb[:])
                nc.sync.dma_start(dbg_h2[:], h2_sb[:])

            # ---- heads: partial dots over this core's 512 h-indices ----
            prodl = pw.tile([1, SH], FD, tag="prodl")
            nc.vector.tensor_mul(prodl[:], h2_sb[:], wld_sb[0:1, 0:SH])
            prodd = pw.tile([1, SH], FD, tag="prodd")
            nc.vector.tensor_mul(prodd[:], h2_sb[:], wld_sb[0:1, SH:2 * SH])
            pd_sb = pw.tile([1, 2], FD, tag="pd")
            nc.vector.tensor_reduce(
                pd_sb[0:1, 0:1], prodl[:], mybir.AxisListType.X,
                mybir.AluOpType.add)
            nc.vector.tensor_reduce(
                pd_sb[0:1, 1:2], prodd[:], mybir.AxisListType.X,
                mybir.AluOpType.add)
            if dbg:
                nc.sync.dma_start(dbg_hd[:], pd_sb[:])

            pd_in = dram.tile([1, 2], FD, tag="pd_in")
            nc.sync.dma_start(pd_in[:], pd_sb[:])
            pd_out = dram.tile([8, 2], FD, tag="pd_out")
            nc.gpsimd.collective_compute(
                "AllGather",
                mybir.AluOpType.bypass,
                replica_groups=[list(range(NC))],
                ins=[pd_in.opt()],
                outs=[pd_out.opt()],
            )
            agp_sb = small.tile([8, 2], FD, tag="agp")
            nc.sync.dma_start(agp_sb[:], pd_out[:])

            psum_f = ppool.tile([2, 1], FD, tag="fin")
            nc.tensor.matmul(
                psum_f[:, :], lhsT=agp_sb[:, :], rhs=ones8[:, :],
                start=True, stop=True,
            )
            fin_sb = pw.tile([2, 1], FD, tag="fin_sb")
            nc.vector.tensor_add(fin_sb[:], psum_f[:], b2_sb[:])
            sig_sb = pw.tile([2, 1], FD, tag="sig_sb")
            nc.scalar.activation(sig_sb[:], fin_sb[:], SIG)
            nc.sync.dma_start(out_l[:], fin_sb[0:1, :])
            nc.sync.dma_start(out_d[:], sig_sb[1:2, :])

    nc.compile()
    return nc


_PROGRAM = None


def _get_program():
    global _PROGRAM
    if _PROGRAM is None:
        _PROGRAM = _build_program(
            dbg=bool(int(os.environ.get("KERNEL_DEBUG", "0"))))
    return _PROGRAM


def make_in_maps(data, h0, c0, w_ih0, w_hh0, b_ih0, b_hh0,
                 w_ih1, w_hh1, b_ih1, b_hh1, wL, bL, wD, bD):
    """Shard + lay out the full inputs for the 8 cores."""
    f32 = np.float32
    data, h0, c0 = (np.asarray(a, f32) for a in (data, h0, c0))
    w_ih0, w_hh0, w_ih1, w_hh1 = (
        np.asarray(a, f32) for a in (w_ih0, w_hh0, w_ih1, w_hh1))
    btot0 = np.asarray(b_ih0, f32) + np.asarray(b_hh0, f32)
    btot1 = np.asarray(b_ih1, f32) + np.asarray(b_hh1, f32)
    wL, bL, wD, bD = (np.asarray(a, f32) for a in (wL, bL, wD, bD))
    wdt = _np_wdt()

    p = np.arange(128)
    # contraction slot (c*128 + p) <-> true index, for partition-major rhs
    ordx = (4 * p[None, :] + np.arange(4)[:, None]).reshape(-1)        # I=512
    ordh = (32 * p[None, :] + np.arange(32)[:, None]).reshape(-1)      # H=4096

    x_c = np.ascontiguousarray(data.reshape(128, 4), dtype=wdt)
    h00_c = np.ascontiguousarray(h0[0, 0].reshape(128, 32), dtype=wdt)
    h01_c = np.ascontiguousarray(h0[1, 0].reshape(128, 32), dtype=wdt)
    b2_c = np.array([[bL[0]], [bD[0]]], f32)

    in_maps = []
    for r in range(NC):
        rows = np.concatenate([g * H + SH * r + np.arange(SH) for g in range(4)])
        sl = slice(SH * r, SH * (r + 1))
        in_maps.append({
            "wih0": np.ascontiguousarray(w_ih0[rows].T[ordx], dtype=wdt),
            "whh0": np.ascontiguousarray(w_hh0[rows].T[ordh], dtype=wdt),
            "whh1": np.ascontiguousarray(w_hh1[rows].T[ordh], dtype=wdt),
            "wih1": np.ascontiguousarray(w_ih1[rows].T[ordh], dtype=wdt),
            "x_in": x_c,
            "h00": h00_c,
            "h01": h01_c,
            "c00": np.ascontiguousarray(c0[0, 0, sl].reshape(1, SH)),
            "c01": np.ascontiguousarray(c0[1, 0, sl].reshape(1, SH)),
            "b0": np.ascontiguousarray(btot0[rows].reshape(1, RJ)),
            "b1": np.ascontiguousarray(btot1[rows].reshape(1, RJ)),
            "wld": np.ascontiguousarray(
                np.concatenate([wL[0, sl], wD[0, sl]]).reshape(1, 2 * SH)),
            "b2": b2_c,
        })
    return in_maps


def kernel(**inputs):
    global LAST_EXEC_NS, LAST_RESULTS
    in_maps = make_in_maps(**inputs)
    nc = _get_program()
    trace = bool(int(os.environ.get("KERNEL_TRACE", "0")))
    res = run_bass_kernel_spmd(
        nc, in_maps, core_ids=list(range(NC)), trace=trace,
    )
    LAST_EXEC_NS = res.exec_time_ns
    LAST_RESULTS = res.results
    r0 = res.results[0]
    d = np.asarray(r0["out_d"], np.float32).reshape(1, 1)
    l = np.asarray(r0["out_l"], np.float32).reshape(1, 1)
    return (d, l)


# revision 19
# speedup vs baseline: 3.3591x; 1.8375x over previous
"""Trainium2 Bass kernel for nn_MimicNetLSTM (2-layer LSTM, H=4096, batch=1, seq=1).

Strategy (tensor-parallel over the 4H gate dim, 8 cores):
  - Core r owns h-indices [512r, 512r+512) of every gate -> 2048 rows of each
    of w_ih0/w_hh0/w_ih1/w_hh1 (~105 MB fp32 per core).  The problem is a
    batch-1 matvec chain, so it is HBM-bandwidth bound: stream weights once.
  - Host pre-arranges each core's weight shard transposed as [K, 2048], rows
    permuted so contraction chunk c / partition p matches the partition-major
    SBUF layout of the activation vector (x/h reshaped [128, K/128]).
  - PE matvec with the ACTIVATION as the stationary operand (1-column
    LDWEIGHTS) and the weight tiles as the moving operand (N=512):
      psum[0:1, n*512:(n+1)*512] += x[:, c].T @ wt[:, n*512:(n+1)*512]
    accumulated over k-chunks c.  Gates land in PSUM partition 0 as
    [1, 2048] = [i | f | g | o] in true h order.
  - LSTM pointwise on DVE/ACT on partition 0.
  - h1 (512 floats/core) is AllGathered between layers (hidden under the
    layer-1 weight stream).  Heads are per-core partial dot products (DVE
    mul + reduce), AllGathered (8 B/core) and summed on every core.
"""

import os
import numpy as np

import concourse.bass as bass
import concourse.tile as tile
from concourse import bacc, mybir
from concourse.bass_utils import run_bass_kernel_spmd

I, H, L = 512, 4096, 2
NC = 8
SH = H // NC          # 512 h-indices per core
RJ = 4 * SH           # 2048 gate rows per core
FD = mybir.dt.float32

# weight dtype on the wire (HBM) + in the matmul. float32 = exact;
# bfloat16 halves DMA bytes (~2x faster) at ~3e-3 relative error.
WEIGHT_DTYPE = os.environ.get("KERNEL_WDT", "float32")
# k-chunks per weight DMA: each dma_start moves DG contiguous MB
DG = int(os.environ.get("KERNEL_DGROUP", "1"))

LAST_EXEC_NS = None
LAST_RESULTS = None


def _wdt():
    return getattr(mybir.dt, WEIGHT_DTYPE)


def _np_wdt():
    if WEIGHT_DTYPE == "float32":
        return np.float32
    import ml_dtypes

    return getattr(ml_dtypes, WEIGHT_DTYPE)


def _build_program(dbg=False):
    nc = bacc.Bacc(
        "TRN2",
        target_bir_lowering=False,
        debug=False,
        enable_asserts=False,
        num_devices=NC,
    )
    wdt = _wdt()

    wih0 = nc.dram_tensor("wih0", [I // DG, DG * RJ], wdt, kind="ExternalInput")
    whh0 = nc.dram_tensor("whh0", [H // DG, DG * RJ], wdt, kind="ExternalInput")
    whh1 = nc.dram_tensor("whh1", [H // DG, DG * RJ], wdt, kind="ExternalInput")
    wih1 = nc.dram_tensor("wih1", [H // DG, DG * RJ], wdt, kind="ExternalInput")
    x_in = nc.dram_tensor("x_in", [128, I // 128], wdt, kind="ExternalInput")
    h00 = nc.dram_tensor("h00", [128, H // 128], wdt, kind="ExternalInput")
    h01 = nc.dram_tensor("h01", [128, H // 128], wdt, kind="ExternalInput")
    c00 = nc.dram_tensor("c00", [1, SH], FD, kind="ExternalInput")
    c01 = nc.dram_tensor("c01", [1, SH], FD, kind="ExternalInput")
    b0 = nc.dram_tensor("b0", [1, RJ], FD, kind="ExternalInput")
    b1 = nc.dram_tensor("b1", [1, RJ], FD, kind="ExternalInput")
    wld = nc.dram_tensor("wld", [1, 2 * SH], FD, kind="ExternalInput")
    b2 = nc.dram_tensor("b2", [2, 1], FD, kind="ExternalInput")
    out_l = nc.dram_tensor("out_l", [1, 1], FD, kind="ExternalOutput")
    out_d = nc.dram_tensor("out_d", [1, 1], FD, kind="ExternalOutput")
    if dbg:
        dbg_g0 = nc.dram_tensor("dbg_g0", [1, RJ], FD, kind="ExternalOutput")
        dbg_h1 = nc.dram_tensor("dbg_h1", [1, SH], FD, kind="ExternalOutput")
        dbg_h1f = nc.dram_tensor("dbg_h1f", [128, 32], FD, kind="ExternalOutput")
        dbg_g1 = nc.dram_tensor("dbg_g1", [1, RJ], FD, kind="ExternalOutput")
        dbg_h2 = nc.dram_tensor("dbg_h2", [1, SH], FD, kind="ExternalOutput")
        dbg_hd = nc.dram_tensor("dbg_hd", [1, 2], FD, kind="ExternalOutput")

    SIG = mybir.ActivationFunctionType.Sigmoid
    TANH = mybir.ActivationFunctionType.Tanh

    wbufs = {1: 8, 2: 7, 4: 4}[DG]
    with tile.TileContext(nc) as tc:
        with (
            tc.tile_pool(name="w", bufs=wbufs) as wpool,
            tc.tile_pool(name="small", bufs=1) as small,
            tc.tile_pool(name="pw", bufs=1) as pw,
            tc.tile_pool(name="psum", bufs=1, space="PSUM") as ppool,
            tc.tile_pool(name="dram", bufs=1, space="DRAM") as dram,
        ):
            def load_small(name, src, shape, dtype=FD):
                t = small.tile(shape, dtype, tag=name)
                nc.sync.dma_start(t[:], src[:])
                return t

            x_sb = load_small("x", x_in, [128, I // 128], wdt)
            h00_sb = load_small("h00", h00, [128, H // 128], wdt)
            h01_sb = load_small("h01", h01, [128, H // 128], wdt)
            c00_sb = load_small("c00", c00, [1, SH])
            c01_sb = load_small("c01", c01, [1, SH])
            b0_sb = load_small("b0", b0, [1, RJ])
            b1_sb = load_small("b1", b1, [1, RJ])
            wld_sb = load_small("wld", wld, [1, 2 * SH])
            b2_sb = load_small("b2", b2, [2, 1])
            ones8 = small.tile([8, 1], FD, tag="ones8")
            nc.vector.memset(ones8[:], 1.0)

            def mm_stream(wdram, rhs_sb, psum, kchunks, first, last):
                for a in range(kchunks // DG):
                    wt = wpool.tile([128, DG * RJ], wdt, tag="w")
                    nc.sync.dma_start(wt[:], wdram[a * 128:(a + 1) * 128, :])
                    for d in range(DG):
                        c = a * DG + d
                        for n in range(4):
                            nc.tensor.matmul(
                                psum[0:1, n * 512:(n + 1) * 512],
                                lhsT=rhs_sb[:, c:c + 1],    # stationary (1 col)
                                rhs=wt[:, d * RJ + n * 512:
                                        d * RJ + (n + 1) * 512],
                                # each n-slice is its own PSUM bank; start
                                # clears the whole bank so set it on the
                                # bank's first MM only
                                start=(first and c == 0),
                                stop=(last and c == kchunks - 1),
                            )

            def pointwise(psum_g, bias_sb, c_sb):
                # everything on partition 0; slices of [1, 2048] = [i|f|g|o]
                gb = pw.tile([1, RJ], FD, tag="gb")
                nc.vector.tensor_add(gb[:], psum_g[0:1, :], bias_sb[:])
                act = pw.tile([1, RJ], FD, tag="act")
                nc.scalar.activation(act[0:1, 0:2 * SH], gb[0:1, 0:2 * SH], SIG)
                nc.scalar.activation(act[0:1, 3 * SH:], gb[0:1, 3 * SH:], SIG)
                nc.scalar.activation(
                    act[0:1, 2 * SH:3 * SH], gb[0:1, 2 * SH:3 * SH], TANH)
                t1 = pw.tile([1, SH], FD, tag="t1")
                nc.vector.tensor_mul(t1[:], act[0:1, SH:2 * SH], c_sb[:])
                t2 = pw.tile([1, SH], FD, tag="t2")
                nc.vector.tensor_mul(
                    t2[:], act[0:1, 0:SH], act[0:1, 2 * SH:3 * SH])
                cn = pw.tile([1, SH], FD, tag="cn")
                nc.vector.tensor_add(cn[:], t1[:], t2[:])
                th = pw.tile([1, SH], FD, tag="th")
                nc.scalar.activation(th[:], cn[:], TANH)
                hn = pw.tile([1, SH], FD, tag="hn")
                nc.vector.tensor_mul(hn[:], act[0:1, 3 * SH:], th[:])
                return hn

            # ---- layer 0 ----
            psum_g0 = ppool.tile([1, RJ], FD, tag="g")
            mm_stream(wih0, x_sb, psum_g0, I // 128, first=True, last=False)
            mm_stream(whh0, h00_sb, psum_g0, H // 128, first=False, last=True)
            h1_sb = pointwise(psum_g0, b0_sb, c00_sb)
            if dbg:
                g0_sb = pw.tile([1, RJ], FD, tag="dbg_g0")
                nc.vector.tensor_copy(g0_sb[:], psum_g0[0:1, :])
                nc.sync.dma_start(dbg_g0[:], g0_sb[:])
                nc.sync.dma_start(dbg_h1[:], h1_sb[:])

            # AllGather h1: 512 floats/core -> 4096 (true h order)
            ag_in = dram.tile([1, SH], FD, tag="ag_in")
            nc.sync.dma_start(ag_in[:], h1_sb[:])
            ag_out = dram.tile([128, 32], FD, tag="ag_out")
            nc.gpsimd.collective_compute(
                "AllGather",
                mybir.AluOpType.bypass,
                replica_groups=[list(range(NC))],
                ins=[ag_in.opt()],
                outs=[ag_out.opt()],
            )
            h1f_sb = small.tile([128, 32], FD, tag="h1f")
            nc.sync.dma_start(h1f_sb[:], ag_out[:])
            if _wdt() != FD:
                h1c_sb = small.tile([128, 32], _wdt(), tag="h1c")
                nc.vector.tensor_copy(h1c_sb[:], h1f_sb[:])
            else:
                h1c_sb = h1f_sb

            # ---- layer 1 ----  (whh1 first: it doesn't depend on the AllGather;
            # psum tag "g" is reused -> waits only for pointwise0's psum read)
            psum_g1 = ppool.tile([1, RJ], FD, tag="g")
            mm_stream(whh1, h01_sb, psum_g1, H // 128, first=True, last=False)
            mm_stream(wih1, h1c_sb, psum_g1, H // 128, first=False, last=True)
            h2_sb = pointwise(psum_g1, b1_sb, c01_sb)
            if dbg:
                nc.sync.dma_start(dbg_h1f[:], h1f_sb[:])
                g1_sb = pw.tile([1, RJ], FD, tag="dbg_g1")
                nc.vector.tensor_copy(g1_sb[:], psum_g1[0:1, :])
                nc.sync.dma_start(dbg_g1[:], g1_sb[:])
                nc.sync.dma_start(dbg_h2[:], h2_sb[:])

            # ---- heads: partial dots over this core's 512 h-indices ----
            prodl = pw.tile([1, SH], FD, tag="prodl")
            nc.vector.tensor_mul(prodl[:], h2_sb[:], wld_sb[0:1, 0:SH])
            prodd = pw.tile([1, SH], FD, tag="prodd")
            nc.vector.tensor_mul(prodd[:], h2_sb[:], wld_sb[0:1, SH:2 * SH])
            pd_sb = pw.tile([1, 2], FD, tag="pd")
            nc.vector.tensor_reduce(
                pd_sb[0:1, 0:1], prodl[:], mybir.AxisListType.X,
                mybir.AluOpType.add)
            nc.vector.tensor_reduce(
                pd_sb[0:1, 1:2], prodd[:], mybir.AxisListType.X,
                mybir.AluOpType.add)
            if dbg:
                nc.sync.dma_start(dbg_hd[:], pd_sb[:])

            pd_in = dram.tile([1, 2], FD, tag="pd_in")
            nc.sync.dma_start(pd_in[:], pd_sb[:])
            pd_out = dram.tile([8, 2], FD, tag="pd_out")
            nc.gpsimd.collective_compute(
                "AllGather",
                mybir.AluOpType.bypass,
                replica_groups=[list(range(NC))],
                ins=[pd_in.opt()],
                outs=[pd_out.opt()],
            )
            agp_sb = small.tile([8, 2], FD, tag="agp")
            nc.sync.dma_start(agp_sb[:], pd_out[:])

            psum_f = ppool.tile([2, 1], FD, tag="fin")
            nc.tensor.matmul(
                psum_f[:, :], lhsT=agp_sb[:, :], rhs=ones8[:, :],
                start=True, stop=True,
            )
            fin_sb = pw.tile([2, 1], FD, tag="fin_sb")
            nc.vector.tensor_add(fin_sb[:], psum_f[:], b2_sb[:])
            sig_sb = pw.tile([2, 1], FD, tag="sig_sb")
            nc.scalar.activation(sig_sb[:], fin_sb[:], SIG)
            nc.sync.dma_start(out_l[:], fin_sb[0:1, :])
            nc.sync.dma_start(out_d[:], sig_sb[1:2, :])

    nc.compile()
    return nc


_PROGRAM = None


def _get_program():
    global _PROGRAM
    if _PROGRAM is None:
        _PROGRAM = _build_program(
            dbg=bool(int(os.environ.get("KERNEL_DEBUG", "0"))))
    return _PROGRAM


def make_in_maps(data, h0, c0, w_ih0, w_hh0, b_ih0, b_hh0,
                 w_ih1, w_hh1, b_ih1, b_hh1, wL, bL, wD, bD):
    """Shard + lay out the full inputs for the 8 cores."""
    f32 = np.float32
    data, h0, c0 = (np.asarray(a, f32) for a in (data, h0, c0))
    w_ih0, w_hh0, w_ih1, w_hh1 = (
        np.asarray(a, f32) for a in (w_ih0, w_hh0, w_ih1, w_hh1))
    btot0 = np.asarray(b_ih0, f32) + np.asarray(b_hh0, f32)
    btot1 = np.asarray(b_ih1, f32) + np.asarray(b_hh1, f32)
    wL, bL, wD, bD = (np.asarray(a, f32) for a in (wL, bL, wD, bD))
    wdt = _np_wdt()

    p = np.arange(128)
    # contraction slot (c*128 + p) <-> true index, for partition-major rhs
    ordx = (4 * p[None, :] + np.arange(4)[:, None]).reshape(-1)        # I=512
    ordh = (32 * p[None, :] + np.arange(32)[:, None]).reshape(-1)      # H=4096

    x_c = np.ascontiguousarray(data.reshape(128, 4), dtype=wdt)
    h00_c = np.ascontiguousarray(h0[0, 0].reshape(128, 32), dtype=wdt)
    h01_c = np.ascontiguousarray(h0[1, 0].reshape(128, 32), dtype=wdt)
    b2_c = np.array([[bL[0]], [bD[0]]], f32)

    def regroup(w):
        # [K, RJ] -> [K//DG, DG*RJ]: one row block = DG k-chunks, so a
        # single dma_start moves DG contiguous MB
        if DG == 1:
            return np.ascontiguousarray(w, dtype=wdt)
        Kd = w.shape[0]
        return np.ascontiguousarray(
            w.reshape(Kd // (128 * DG), DG, 128, RJ)
            .transpose(0, 2, 1, 3).reshape(Kd // DG, DG * RJ), dtype=wdt)

    in_maps = []
    for r in range(NC):
        rows = np.concatenate([g * H + SH * r + np.arange(SH) for g in range(4)])
        sl = slice(SH * r, SH * (r + 1))
        in_maps.append({
            "wih0": regroup(w_ih0[rows].T[ordx]),
            "whh0": regroup(w_hh0[rows].T[ordh]),
            "whh1": regroup(w_hh1[rows].T[ordh]),
            "wih1": regroup(w_ih1[rows].T[ordh]),
            "x_in": x_c,
            "h00": h00_c,
            "h01": h01_c,
            "c00": np.ascontiguousarray(c0[0, 0, sl].reshape(1, SH)),
            "c01": np.ascontiguousarray(c0[1, 0, sl].reshape(1, SH)),
            "b0": np.ascontiguousarray(btot0[rows].reshape(1, RJ)),
            "b1": np.ascontiguousarray(btot1[rows].reshape(1, RJ)),
            "wld": np.ascontiguousarray(
                np.concatenate([wL[0, sl], wD[0, sl]]).reshape(1, 2 * SH)),
            "b2": b2_c,
        })
    return in_maps


def kernel(**inputs):
    global LAST_EXEC_NS, LAST_RESULTS
    in_maps = make_in_maps(**inputs)
    nc = _get_program()
    trace = bool(int(os.environ.get("KERNEL_TRACE", "0")))
    res = run_bass_kernel_spmd(
        nc, in_maps, core_ids=list(range(NC)), trace=trace,
    )
    LAST_EXEC_NS = res.exec_time_ns
    LAST_RESULTS = res.results
    r0 = res.results[0]
    d = np.asarray(r0["out_d"], np.float32).reshape(1, 1)
    l = np.asarray(r0["out_l"], np.float32).reshape(1, 1)
    return (d, l)


# revision 20
# speedup vs baseline: 3.3836x; 1.0073x over previous
"""Trainium2 Bass kernel for nn_MimicNetLSTM (2-layer LSTM, H=4096, batch=1, seq=1).

Strategy (tensor-parallel over the 4H gate dim, 8 cores):
  - Core r owns h-indices [512r, 512r+512) of every gate -> 2048 rows of each
    of w_ih0/w_hh0/w_ih1/w_hh1 (~105 MB fp32 per core).  The problem is a
    batch-1 matvec chain, so it is HBM-bandwidth bound: stream weights once.
  - Host pre-arranges each core's weight shard transposed as [K, 2048], rows
    permuted so contraction chunk c / partition p matches the partition-major
    SBUF layout of the activation vector (x/h reshaped [128, K/128]).
  - PE matvec with the ACTIVATION as the stationary operand (1-column
    LDWEIGHTS) and the weight tiles as the moving operand (N=512):
      psum[0:1, n*512:(n+1)*512] += x[:, c].T @ wt[:, n*512:(n+1)*512]
    accumulated over k-chunks c.  Gates land in PSUM partition 0 as
    [1, 2048] = [i | f | g | o] in true h order.
  - LSTM pointwise on DVE/ACT on partition 0.
  - h1 (512 floats/core) is AllGathered between layers (hidden under the
    layer-1 weight stream).  Heads are per-core partial dot products (DVE
    mul + reduce), AllGathered (8 B/core) and summed on every core.
"""

import os
import numpy as np

import concourse.bass as bass
import concourse.tile as tile
from concourse import bacc, mybir
from concourse.bass_utils import run_bass_kernel_spmd

I, H, L = 512, 4096, 2
NC = 8
SH = H // NC          # 512 h-indices per core
RJ = 4 * SH           # 2048 gate rows per core
FD = mybir.dt.float32

# weight dtype on the wire (HBM) + in the matmul. float32 = exact;
# bfloat16 halves DMA bytes (~2x faster) at ~3e-3 relative error.
WEIGHT_DTYPE = os.environ.get("KERNEL_WDT", "float32")
# k-chunks per weight DMA: each dma_start moves DG contiguous MB
DG = int(os.environ.get("KERNEL_DGROUP", "1"))

LAST_EXEC_NS = None
LAST_RESULTS = None


def _wdt():
    return getattr(mybir.dt, WEIGHT_DTYPE)


def _np_wdt():
    if WEIGHT_DTYPE == "float32":
        return np.float32
    if WEIGHT_DTYPE == "float16":
        return np.float16
    import ml_dtypes

    return getattr(ml_dtypes, WEIGHT_DTYPE)


def _build_program(dbg=False):
    nc = bacc.Bacc(
        "TRN2",
        target_bir_lowering=False,
        debug=False,
        enable_asserts=False,
        num_devices=NC,
    )
    wdt = _wdt()

    wih0 = nc.dram_tensor("wih0", [I // DG, DG * RJ], wdt, kind="ExternalInput")
    whh0 = nc.dram_tensor("whh0", [H // DG, DG * RJ], wdt, kind="ExternalInput")
    whh1 = nc.dram_tensor("whh1", [H // DG, DG * RJ], wdt, kind="ExternalInput")
    wih1 = nc.dram_tensor("wih1", [H // DG, DG * RJ], wdt, kind="ExternalInput")
    x_in = nc.dram_tensor("x_in", [128, I // 128], wdt, kind="ExternalInput")
    h00 = nc.dram_tensor("h00", [128, H // 128], wdt, kind="ExternalInput")
    h01 = nc.dram_tensor("h01", [128, H // 128], wdt, kind="ExternalInput")
    c00 = nc.dram_tensor("c00", [1, SH], FD, kind="ExternalInput")
    c01 = nc.dram_tensor("c01", [1, SH], FD, kind="ExternalInput")
    b0 = nc.dram_tensor("b0", [1, RJ], FD, kind="ExternalInput")
    b1 = nc.dram_tensor("b1", [1, RJ], FD, kind="ExternalInput")
    wld = nc.dram_tensor("wld", [1, 2 * SH], FD, kind="ExternalInput")
    b2 = nc.dram_tensor("b2", [2, 1], FD, kind="ExternalInput")
    out_l = nc.dram_tensor("out_l", [1, 1], FD, kind="ExternalOutput")
    out_d = nc.dram_tensor("out_d", [1, 1], FD, kind="ExternalOutput")
    if dbg:
        dbg_g0 = nc.dram_tensor("dbg_g0", [1, RJ], FD, kind="ExternalOutput")
        dbg_h1 = nc.dram_tensor("dbg_h1", [1, SH], FD, kind="ExternalOutput")
        dbg_h1f = nc.dram_tensor("dbg_h1f", [128, 32], FD, kind="ExternalOutput")
        dbg_g1 = nc.dram_tensor("dbg_g1", [1, RJ], FD, kind="ExternalOutput")
        dbg_h2 = nc.dram_tensor("dbg_h2", [1, SH], FD, kind="ExternalOutput")
        dbg_hd = nc.dram_tensor("dbg_hd", [1, 2], FD, kind="ExternalOutput")

    SIG = mybir.ActivationFunctionType.Sigmoid
    TANH = mybir.ActivationFunctionType.Tanh

    wbufs = {1: 8, 2: 7, 4: 4}[DG]
    with tile.TileContext(nc) as tc:
        with (
            tc.tile_pool(name="w", bufs=wbufs) as wpool,
            tc.tile_pool(name="small", bufs=1) as small,
            tc.tile_pool(name="pw", bufs=1) as pw,
            tc.tile_pool(name="psum", bufs=1, space="PSUM") as ppool,
            tc.tile_pool(name="dram", bufs=1, space="DRAM") as dram,
        ):
            def load_small(name, src, shape, dtype=FD):
                t = small.tile(shape, dtype, tag=name)
                nc.sync.dma_start(t[:], src[:])
                return t

            x_sb = load_small("x", x_in, [128, I // 128], wdt)
            h00_sb = load_small("h00", h00, [128, H // 128], wdt)
            h01_sb = load_small("h01", h01, [128, H // 128], wdt)
            c00_sb = load_small("c00", c00, [1, SH])
            c01_sb = load_small("c01", c01, [1, SH])
            b0_sb = load_small("b0", b0, [1, RJ])
            b1_sb = load_small("b1", b1, [1, RJ])
            wld_sb = load_small("wld", wld, [1, 2 * SH])
            b2_sb = load_small("b2", b2, [2, 1])
            ones8 = small.tile([8, 1], FD, tag="ones8")
            nc.vector.memset(ones8[:], 1.0)

            def mm_stream(wdram, rhs_sb, psum, kchunks, first, last):
                for a in range(kchunks // DG):
                    wt = wpool.tile([128, DG * RJ], wdt, tag="w")
                    nc.sync.dma_start(wt[:], wdram[a * 128:(a + 1) * 128, :])
                    for d in range(DG):
                        c = a * DG + d
                        for n in range(4):
                            nc.tensor.matmul(
                                psum[0:1, n * 512:(n + 1) * 512],
                                lhsT=rhs_sb[:, c:c + 1],    # stationary (1 col)
                                rhs=wt[:, d * RJ + n * 512:
                                        d * RJ + (n + 1) * 512],
                                # each n-slice is its own PSUM bank; start
                                # clears the whole bank so set it on the
                                # bank's first MM only
                                start=(first and c == 0),
                                stop=(last and c == kchunks - 1),
                            )

            def pointwise(psum_g, bias_sb, c_sb):
                # everything on partition 0; slices of [1, 2048] = [i|f|g|o]
                gb = pw.tile([1, RJ], FD, tag="gb")
                nc.vector.tensor_add(gb[:], psum_g[0:1, :], bias_sb[:])
                act = pw.tile([1, RJ], FD, tag="act")
                nc.scalar.activation(act[0:1, 0:2 * SH], gb[0:1, 0:2 * SH], SIG)
                nc.scalar.activation(act[0:1, 3 * SH:], gb[0:1, 3 * SH:], SIG)
                nc.scalar.activation(
                    act[0:1, 2 * SH:3 * SH], gb[0:1, 2 * SH:3 * SH], TANH)
                t1 = pw.tile([1, SH], FD, tag="t1")
                nc.vector.tensor_mul(t1[:], act[0:1, SH:2 * SH], c_sb[:])
                t2 = pw.tile([1, SH], FD, tag="t2")
                nc.vector.tensor_mul(
                    t2[:], act[0:1, 0:SH], act[0:1, 2 * SH:3 * SH])
                cn = pw.tile([1, SH], FD, tag="cn")
                nc.vector.tensor_add(cn[:], t1[:], t2[:])
                th = pw.tile([1, SH], FD, tag="th")
                nc.scalar.activation(th[:], cn[:], TANH)
                hn = pw.tile([1, SH], FD, tag="hn")
                nc.vector.tensor_mul(hn[:], act[0:1, 3 * SH:], th[:])
                return hn

            # ---- layer 0 ----
            psum_g0 = ppool.tile([1, RJ], FD, tag="g")
            mm_stream(wih0, x_sb, psum_g0, I // 128, first=True, last=False)
            mm_stream(whh0, h00_sb, psum_g0, H // 128, first=False, last=True)
            h1_sb = pointwise(psum_g0, b0_sb, c00_sb)
            if dbg:
                g0_sb = pw.tile([1, RJ], FD, tag="dbg_g0")
                nc.vector.tensor_copy(g0_sb[:], psum_g0[0:1, :])
                nc.sync.dma_start(dbg_g0[:], g0_sb[:])
                nc.sync.dma_start(dbg_h1[:], h1_sb[:])

            # AllGather h1: 512 floats/core -> 4096 (true h order)
            ag_in = dram.tile([1, SH], FD, tag="ag_in")
            nc.sync.dma_start(ag_in[:], h1_sb[:])
            ag_out = dram.tile([128, 32], FD, tag="ag_out")
            nc.gpsimd.collective_compute(
                "AllGather",
                mybir.AluOpType.bypass,
                replica_groups=[list(range(NC))],
                ins=[ag_in.opt()],
                outs=[ag_out.opt()],
            )
            h1f_sb = small.tile([128, 32], FD, tag="h1f")
            nc.sync.dma_start(h1f_sb[:], ag_out[:])
            if _wdt() != FD:
                h1c_sb = small.tile([128, 32], _wdt(), tag="h1c")
                nc.vector.tensor_copy(h1c_sb[:], h1f_sb[:])
            else:
                h1c_sb = h1f_sb

            # ---- layer 1 ----  (whh1 first: it doesn't depend on the AllGather;
            # psum tag "g" is reused -> waits only for pointwise0's psum read)
            psum_g1 = ppool.tile([1, RJ], FD, tag="g")
            mm_stream(whh1, h01_sb, psum_g1, H // 128, first=True, last=False)
            mm_stream(wih1, h1c_sb, psum_g1, H // 128, first=False, last=True)
            h2_sb = pointwise(psum_g1, b1_sb, c01_sb)
            if dbg:
                nc.sync.dma_start(dbg_h1f[:], h1f_sb[:])
                g1_sb = pw.tile([1, RJ], FD, tag="dbg_g1")
                nc.vector.tensor_copy(g1_sb[:], psum_g1[0:1, :])
                nc.sync.dma_start(dbg_g1[:], g1_sb[:])
                nc.sync.dma_start(dbg_h2[:], h2_sb[:])

            # ---- heads: partial dots over this core's 512 h-indices ----
            prodl = pw.tile([1, SH], FD, tag="prodl")
            nc.vector.tensor_mul(prodl[:], h2_sb[:], wld_sb[0:1, 0:SH])
            prodd = pw.tile([1, SH], FD, tag="prodd")
            nc.vector.tensor_mul(prodd[:], h2_sb[:], wld_sb[0:1, SH:2 * SH])
            pd_sb = pw.tile([1, 2], FD, tag="pd")
            nc.vector.tensor_reduce(
                pd_sb[0:1, 0:1], prodl[:], mybir.AxisListType.X,
                mybir.AluOpType.add)
            nc.vector.tensor_reduce(
                pd_sb[0:1, 1:2], prodd[:], mybir.AxisListType.X,
                mybir.AluOpType.add)
            if dbg:
                nc.sync.dma_start(dbg_hd[:], pd_sb[:])

            pd_in = dram.tile([1, 2], FD, tag="pd_in")
            nc.sync.dma_start(pd_in[:], pd_sb[:])
            pd_out = dram.tile([8, 2], FD, tag="pd_out")
            nc.gpsimd.collective_compute(
                "AllGather",
                mybir.AluOpType.bypass,
                replica_groups=[list(range(NC))],
                ins=[pd_in.opt()],
                outs=[pd_out.opt()],
            )
            agp_sb = small.tile([8, 2], FD, tag="agp")
            nc.sync.dma_start(agp_sb[:], pd_out[:])

            psum_f = ppool.tile([2, 1], FD, tag="fin")
            nc.tensor.matmul(
                psum_f[:, :], lhsT=agp_sb[:, :], rhs=ones8[:, :],
                start=True, stop=True,
            )
            fin_sb = pw.tile([2, 1], FD, tag="fin_sb")
            nc.vector.tensor_add(fin_sb[:], psum_f[:], b2_sb[:])
            sig_sb = pw.tile([2, 1], FD, tag="sig_sb")
            nc.scalar.activation(sig_sb[:], fin_sb[:], SIG)
            nc.sync.dma_start(out_l[:], fin_sb[0:1, :])
            nc.sync.dma_start(out_d[:], sig_sb[1:2, :])

    nc.compile()
    return nc


_PROGRAM = None


def _get_program():
    global _PROGRAM
    if _PROGRAM is None:
        _PROGRAM = _build_program(
            dbg=bool(int(os.environ.get("KERNEL_DEBUG", "0"))))
    return _PROGRAM


def make_in_maps(data, h0, c0, w_ih0, w_hh0, b_ih0, b_hh0,
                 w_ih1, w_hh1, b_ih1, b_hh1, wL, bL, wD, bD):
    """Shard + lay out the full inputs for the 8 cores."""
    f32 = np.float32
    data, h0, c0 = (np.asarray(a, f32) for a in (data, h0, c0))
    w_ih0, w_hh0, w_ih1, w_hh1 = (
        np.asarray(a, f32) for a in (w_ih0, w_hh0, w_ih1, w_hh1))
    btot0 = np.asarray(b_ih0, f32) + np.asarray(b_hh0, f32)
    btot1 = np.asarray(b_ih1, f32) + np.asarray(b_hh1, f32)
    wL, bL, wD, bD = (np.asarray(a, f32) for a in (wL, bL, wD, bD))
    wdt = _np_wdt()

    p = np.arange(128)
    # contraction slot (c*128 + p) <-> true index, for partition-major rhs
    ordx = (4 * p[None, :] + np.arange(4)[:, None]).reshape(-1)        # I=512
    ordh = (32 * p[None, :] + np.arange(32)[:, None]).reshape(-1)      # H=4096

    x_c = np.ascontiguousarray(data.reshape(128, 4), dtype=wdt)
    h00_c = np.ascontiguousarray(h0[0, 0].reshape(128, 32), dtype=wdt)
    h01_c = np.ascontiguousarray(h0[1, 0].reshape(128, 32), dtype=wdt)
    b2_c = np.array([[bL[0]], [bD[0]]], f32)

    def regroup(w):
        # [K, RJ] -> [K//DG, DG*RJ]: one row block = DG k-chunks, so a
        # single dma_start moves DG contiguous MB
        if DG == 1:
            return np.ascontiguousarray(w, dtype=wdt)
        Kd = w.shape[0]
        return np.ascontiguousarray(
            w.reshape(Kd // (128 * DG), DG, 128, RJ)
            .transpose(0, 2, 1, 3).reshape(Kd // DG, DG * RJ), dtype=wdt)

    in_maps = []
    for r in range(NC):
        rows = np.concatenate([g * H + SH * r + np.arange(SH) for g in range(4)])
        sl = slice(SH * r, SH * (r + 1))
        in_maps.append({
            "wih0": regroup(w_ih0[rows].T[ordx]),
            "whh0": regroup(w_hh0[rows].T[ordh]),
            "whh1": regroup(w_hh1[rows].T[ordh]),
            "wih1": regroup(w_ih1[rows].T[ordh]),
            "x_in": x_c,
            "h00": h00_c,
            "h01": h01_c,
            "c00": np.ascontiguousarray(c0[0, 0, sl].reshape(1, SH)),
            "c01": np.ascontiguousarray(c0[1, 0, sl].reshape(1, SH)),
            "b0": np.ascontiguousarray(btot0[rows].reshape(1, RJ)),
            "b1": np.ascontiguousarray(btot1[rows].reshape(1, RJ)),
            "wld": np.ascontiguousarray(
                np.concatenate([wL[0, sl], wD[0, sl]]).reshape(1, 2 * SH)),
            "b2": b2_c,
        })
    return in_maps


def kernel(**inputs):
    global LAST_EXEC_NS, LAST_RESULTS
    in_maps = make_in_maps(**inputs)
    nc = _get_program()
    trace = bool(int(os.environ.get("KERNEL_TRACE", "0")))
    res = run_bass_kernel_spmd(
        nc, in_maps, core_ids=list(range(NC)), trace=trace,
    )
    LAST_EXEC_NS = res.exec_time_ns
    LAST_RESULTS = res.results
    r0 = res.results[0]
    d = np.asarray(r0["out_d"], np.float32).reshape(1, 1)
    l = np.asarray(r0["out_l"], np.float32).reshape(1, 1)
    return (d, l)
